# revision 12
# baseline (speedup 1.0000x reference)
"""Bass/Trainium2 kernel for 3-layer GAT (nn_GAT_90744069030460).

Strategy (8-core SPMD, graph/data parallel):
  - Nodes sharded contiguously across 8 cores by dst owner.  Each core's dst
    nodes are assigned to 64-slot "windows" (host balances per-src-range edge
    loads across windows).
  - Per layer a packed gather table (256B-pitch rows: [h bf16 x fo | a_src
    f32 | pad]) is built for all nodes: layer 1 computes it locally from the
    replicated x; layers 2-3 compute own rows, AllGather a compact copy, and
    re-pitch locally.
  - Edge phase: edges grouped by (super-chunk of 4 windows) x (src range).
    Ranges are owner-pairs so int16 dma_gather indices reach.  One bulk
    dma_gather per (super-chunk, range) fetches h+a_src rows; a second 4-byte
    dma_gather fetches a_dst per edge.  Per-edge p = exp(leakyrelu(a_src +
    a_dst)) (segment-max-free softmax, safe in f32).  Segment sums go through
    one-hot selection matmuls into per-window PSUM accumulators; the softmax
    denominator comes from an extra column holding p.
  - Normalize (divide by denominator) + bias + relu in bulk on DVE.

Host-side numpy does all graph prep (window balancing, padding, index
arrays); the device only runs dense/regular work.
"""

import sys

sys.path.insert(0, "/opt/trn_rl_repo")

import heapq
import numpy as np

N_CORES = 8
WSL = 64   # dst slots per window
PGA = 4    # phase-A blocks per group
SCW = 4    # windows per super-chunk
NRANGE = 4  # src ranges (owner pairs)

_cache = {}


# --------------------------------------------------------------------------
# Host-side graph preprocessing
# --------------------------------------------------------------------------

def _balance_windows(degv, nw):
    """Assign local dsts to (window, pos) slots.  degv: [d_own, NRANGE] edge
    counts per src range.  Balances the per-range max load across windows,
    capacity WSL dsts per window."""
    d_own = degv.shape[0]
    tot = degv.sum(axis=1)
    order = np.argsort(-tot, kind="stable")
    loads = np.zeros((nw, NRANGE), dtype=np.int64)
    counts = np.zeros(nw, dtype=np.int64)
    slot = np.empty(d_own, dtype=np.int64)
    target = np.maximum(degv.sum(axis=0) / nw, 1.0)
    for i in order:
        cand = ((loads + degv[i][None, :]) / target[None, :]).max(axis=1)
        cand[counts >= WSL] = np.inf
        w = int(np.argmin(cand))
        slot[i] = w * WSL + counts[w]
        counts[w] += 1
        loads[w] += degv[i]
    return slot


def _prep(x, edge_index):
    n, f_in = x.shape
    assert n % N_CORES == 0
    d_own = n // N_CORES
    nb = (d_own + 127) // 128
    d_pad = nb * 128
    nw = 2 * nb
    nbg = N_CORES * nb
    npadg = N_CORES * d_pad
    range_rows = 2 * d_pad
    assert range_rows <= 32768

    src = np.concatenate([edge_index[0].astype(np.int64), np.arange(n, dtype=np.int64)])
    dst = np.concatenate([edge_index[1].astype(np.int64), np.arange(n, dtype=np.int64)])
    owner = dst // d_own
    loc = dst - owner * d_own
    src_owner = src // d_own
    src_range = src_owner // 2

    slot_of = np.empty(n, dtype=np.int64)
    for c in range(N_CORES):
        m = owner == c
        degv = np.zeros((d_own, NRANGE), dtype=np.int64)
        np.add.at(degv, (loc[m], src_range[m]), 1)
        slot_of[c * d_own : (c + 1) * d_own] = _balance_windows(degv, nw)

    s_e = slot_of[dst]
    w_e = s_e // WSL

    # per-range K (tiles per (window, range)), global max across cores
    K = np.zeros(NRANGE, dtype=np.int64)
    percore = []
    for c in range(N_CORES):
        idx = np.nonzero(owner == c)[0]
        cnt = np.zeros((nw, NRANGE), dtype=np.int64)
        np.add.at(cnt, (w_e[idx], src_range[idx]), 1)
        K = np.maximum(K, (cnt.max(axis=0) + 127) // 128)
        percore.append((idx, cnt))
    K = np.maximum(K, 0)
    cumK = np.zeros(NRANGE + 1, dtype=np.int64)
    cumK[1:] = np.cumsum(K)
    k_tot = int(cumK[-1])
    scw = min(SCW, nw)
    assert nw % scw == 0
    T = nw * k_tot

    # table row of node (o, s): r = o*d_pad + (s%128)*nb + s//128
    s_src = slot_of[src]
    row_src = src_owner * d_pad + (s_src % 128) * nb + s_src // 128
    rel_src = row_src - src_range * range_rows
    assert rel_src.min() >= 0 and rel_src.max() < range_rows

    cores = []
    for c in range(N_CORES):
        idx, cnt = percore[c]
        we_ = w_e[idx]
        se_ = src_range[idx]
        order = np.argsort(we_ * NRANGE + se_, kind="stable")
        idx = idx[order]
        we_ = we_[order]
        se_ = se_[order]
        # rank within (window, range)
        starts = np.zeros(nw * NRANGE, dtype=np.int64)
        flatcnt = cnt.reshape(-1)
        starts[1:] = np.cumsum(flatcnt)[:-1]
        rank = np.arange(idx.shape[0], dtype=np.int64) - starts[we_ * NRANGE + se_]
        t = ((we_ // scw) * (scw * k_tot) + scw * cumK[se_]
             + (we_ % scw) * K[se_] + rank // 128)
        p = rank % 128

        src16 = np.zeros((128, 8 * T), dtype=np.int16)
        aidx16 = np.zeros((128, 8 * T), dtype=np.int16)
        dstrel = np.full((128, T), float(WSL), dtype=np.float32)

        sev = s_e[idx]
        aval = ((sev % 128) * nb + sev // 128).astype(np.int16)
        sval = rel_src[idx].astype(np.int16)
        col = t * 8 + p // 16
        prow = (p % 16).astype(np.int64)
        for r in range(8):
            src16[16 * r + prow, col] = sval
            aidx16[16 * r + prow, col] = aval
        dstrel[p, t] = (sev % WSL).astype(np.float32)
        cores.append(dict(src16=src16, aidx16=aidx16, dstrel=dstrel))

    # node_at[o, slot]
    node_at = np.full((N_CORES, d_pad), -1, dtype=np.int64)
    allnodes = np.arange(n, dtype=np.int64)
    node_at[allnodes // d_own, slot_of] = allnodes

    # xT [f_in, npadg]: col (o*nb+b)*128+q = x[node_at[o, b*128+q]]
    j = np.arange(nbg)
    q = np.arange(128)
    o_j = (j // nb)[:, None]
    s_jq = ((j % nb) * 128)[:, None] + q[None, :]
    nid = node_at[o_j, s_jq]
    x_rows = np.where(nid[:, :, None] >= 0, x[np.clip(nid, 0, None), :], 0.0)
    xT = np.ascontiguousarray(x_rows.reshape(nbg * 128, f_in).T.astype(np.float32))

    # per-core xT_own [f_in, d_pad]
    xT_owns = []
    for c in range(N_CORES):
        nid = node_at[c]
        xr = np.where(nid[:, None] >= 0, x[np.clip(nid, 0, None), :], 0.0)
        xT_owns.append(np.ascontiguousarray(xr.T.astype(np.float32)))

    meta = dict(n=n, f_in=f_in, d_own=d_own, nb=nb, d_pad=d_pad, nw=nw,
                nbg=nbg, npadg=npadg, range_rows=range_rows, scw=scw,
                K=tuple(int(v) for v in K), k_tot=k_tot, t_tiles=T)
    return meta, cores, xT, xT_owns, slot_of


def _make_wext(W, a_s, a_d):
    f_in, f_out = W.shape
    m = np.zeros((f_in, f_out + 4), dtype=np.float32)
    m[:, 0:f_out] = W
    m[:, f_out] = W @ a_s
    m[:, f_out + 1] = W @ a_d
    return m


# --------------------------------------------------------------------------
# Device program
# --------------------------------------------------------------------------

def _dma_gather_raw(eng, out_ap, in_ap, idxs_ap, num_idxs, elem_size,
                    elem_step, queue_num):
    """bass.dma_gather (non-transpose, DRAM source) without the 256B
    elem-size assert — the Q7 ucode only needs the row *stride* to be a
    256B multiple; elem_size is arbitrary (packets are min(elem, 16K))."""
    import concourse.bass as bass
    import concourse.mybir as mybir
    from concourse.bass import MemorySpace
    from concourse._compat import exact_div

    assert idxs_ap.dtype == mybir.dt.int16
    assert in_ap.space == MemorySpace.DRAM
    assert in_ap.dtype == out_ap.dtype
    assert in_ap.ap[-1][1] == out_ap.ap[-1][1] == elem_size
    assert in_ap.ap[0][0] == elem_step
    assert num_idxs % 128 == 0
    assert out_ap.ap[0][1] * out_ap.ap[1][1] == num_idxs
    stride_bytes = elem_step * mybir.dt.size(in_ap.dtype)
    stride_bytes_256 = exact_div(stride_bytes, 256)
    assert 0 < stride_bytes_256 < 256
    _in_ap = eng.lower_ap_dma(in_ap, for_custom_bir_dma=True)
    _idxs_ap = eng.lower_ap(idxs_ap)
    _out_ap = eng.lower_ap(out_ap)
    return eng.add_instruction(
        mybir.InstDMAGatherAnt(
            name=eng.bass.get_next_instruction_name(),
            ins=[*_in_ap, _idxs_ap,
                 eng.lower_val_access(eng.to_reg(num_idxs))],
            outs=[_out_ap],
            transpose=False,
            num_idxs=num_idxs,
            elem_size=elem_size,
            stride_bytes_256=stride_bytes_256,
            gen_mode=0,
            single_packet=True,
            queue_num=queue_num,
            sbuf_tokens_per_rank=0,
            sbuf_free_dim_per_rank=0,
            sbuf_free_dim_pad_per_rank=0,
            sbuf_byte_offset=0,
        )
    )


def _build_program(meta, num_cores, debug=False):
    import concourse.bass as bass
    import concourse.bacc as bacc
    import concourse.tile as tile
    import concourse.mybir as mybir
    from concourse.masks import make_identity

    f32 = mybir.dt.float32
    bf16 = mybir.dt.bfloat16
    i16 = mybir.dt.int16
    OP = mybir.AluOpType
    AF = mybir.ActivationFunctionType

    nb = meta["nb"]
    nbg = meta["nbg"]
    npadg = meta["npadg"]
    d_pad = meta["d_pad"]
    nw = meta["nw"]
    rng_rows = meta["range_rows"]
    scw = meta["scw"]
    K = meta["K"]
    k_tot = meta["k_tot"]
    T = meta["t_tiles"]
    f_in0 = meta["f_in"]
    cumK = [0]
    for v in K:
        cumK.append(cumK[-1] + v)

    # layer params: he = packed row cols used (int16), rw = matmul rhs width
    LAYERS = [
        dict(f_in=f_in0, f_out=64, relu=True),
        dict(f_in=64, f_out=64, relu=True),
        dict(f_in=64, f_out=32, relu=False),
    ]
    for L in LAYERS:
        L["he"] = L["f_out"] + 2      # h bf16 cols + a_src f32 (2 int16 cols)
        L["rw"] = L["f_out"] + 1      # rhs: p*h cols + p col (denominator)
        L["we"] = L["f_out"] + 4      # W_ext: W | W@as | W@ad | pad

    nc = bacc.Bacc("TRN2", target_bir_lowering=False, debug=debug,
                   num_devices=num_cores)

    # ---- I/O ----
    xT_in = nc.dram_tensor("xT", [f_in0, npadg], f32, kind="ExternalInput")
    xTo_in = nc.dram_tensor("xTo", [f_in0, d_pad], f32, kind="ExternalInput")
    src16_in = nc.dram_tensor("src16", [128, 8 * T], i16, kind="ExternalInput")
    aidx16_in = nc.dram_tensor("aidx16", [128, 8 * T], i16, kind="ExternalInput")
    dstrel_in = nc.dram_tensor("dstrel", [128, T], f32, kind="ExternalInput")
    iota_in = nc.dram_tensor("iota", [128, WSL], f32, kind="ExternalInput")
    w_in = {li: nc.dram_tensor(f"wext{li}", [L["f_in"], L["we"]], f32,
                               kind="ExternalInput")
            for li, L in enumerate(LAYERS)}
    b_in = {li: nc.dram_tensor(f"bias{li}", [128, LAYERS[li]["f_out"]], f32,
                               kind="ExternalInput") for li in range(3)}
    outs = {li: nc.dram_tensor(f"out{li}", [128, nb * LAYERS[li]["rw"]], f32,
                               kind="ExternalOutput") for li in range(3)}

    rg = [list(range(num_cores))]

    with tile.TileContext(nc) as tc:
        with (
            tc.tile_pool(name="const", bufs=1) as constp,
            tc.tile_pool(name="edat", bufs=1) as edat,
            tc.tile_pool(name="res", bufs=1) as resp,
            tc.tile_pool(name="stage", bufs=3) as stage,
            tc.tile_pool(name="pa", bufs=3) as pa,
            tc.tile_pool(name="pw", bufs=4, space="PSUM") as psum_w,
            tc.tile_pool(name="pah", bufs=2, space="PSUM") as psum_a,
            tc.tile_pool(name="dram", bufs=1, space="DRAM") as dramp,
        ):
            ident = constp.tile([128, 128], f32, name="ident")
            make_identity(nc, ident[:])
            iota_sb = constp.tile([128, WSL], f32, name="iota_sb")
            nc.sync.dma_start(out=iota_sb[:], in_=iota_in[:, :])
            wsb = {}
            bsb = {}
            for li, L in enumerate(LAYERS):
                wsb[li] = constp.tile([L["f_in"], L["we"]], f32, name=f"w{li}")
                nc.sync.dma_start(out=wsb[li][:], in_=w_in[li][:, :])
                bsb[li] = constp.tile([128, L["f_out"]], f32, name=f"b{li}")
                nc.sync.dma_start(out=bsb[li][:], in_=b_in[li][:, :])

            src16 = edat.tile([128, 8 * T], i16, name="src16")
            nc.sync.dma_start(out=src16[:], in_=src16_in[:, :])
            aidx16 = edat.tile([128, 8 * T], i16, name="aidx16")
            nc.sync.dma_start(out=aidx16[:], in_=aidx16_in[:, :])
            dstrel = edat.tile([128, T], f32, name="dstrel")
            nc.sync.dma_start(out=dstrel[:], in_=dstrel_in[:, :])

            # ---- DRAM scratch ----
            tbl = dramp.tile([npadg, 128], i16, name="tbl")  # packed gather table
            hxo_c = {1: dramp.tile([d_pad, LAYERS[1]["he"]], i16, name="hxo2"),
                     2: dramp.tile([d_pad, LAYERS[2]["he"]], i16, name="hxo3")}
            hxf_c = {1: dramp.tile([npadg, LAYERS[1]["he"]], i16, name="hxf2"),
                     2: dramp.tile([npadg, LAYERS[2]["he"]], i16, name="hxf3")}
            adst_d = {li: dramp.tile([d_pad, 64], f32, name=f"adst{li}")
                      for li in range(3)}

            res_t = {}

            def alloc_res(li):
                res_t[li] = resp.tile([128, nb * LAYERS[li]["rw"]], f32,
                                      name=f"res{li}", tag="res",
                                      padded_shape=[128, nb * 65])
                return res_t[li]

            # ------------- phase A: layer 1 (all nodes -> pitched tbl) -----
            L = LAYERS[0]
            he, we, fo = L["he"], L["we"], L["f_out"]
            tbl_v = tbl[:].rearrange("(o p b) c -> o p b c", o=num_cores, p=128)
            for o in range(num_cores):
                for g0 in range(0, nb, PGA):
                    gsz = min(PGA, nb - g0)
                    xt = pa.tile([f_in0, PGA * 128], f32, name="xt", tag="xt")
                    nc.sync.dma_start(
                        out=xt[:, 0 : gsz * 128],
                        in_=xT_in[:, (o * nb + g0) * 128 : (o * nb + g0 + gsz) * 128])
                    st = pa.tile([128, PGA * he], i16, name="st1", tag="st1")
                    st_bf = st[:].bitcast(bf16).rearrange("p (j c) -> p j c", c=he)
                    st_f32 = st[:].bitcast(f32).rearrange("p (j c) -> p j c",
                                                          c=he // 2)
                    for k in range(gsz):
                        ps = psum_a.tile([128, we], f32, name="pa_h", tag="pa_h")
                        nc.tensor.matmul(out=ps[:],
                                         lhsT=xt[:, k * 128 : (k + 1) * 128],
                                         rhs=wsb[0][:], start=True, stop=True)
                        nc.vector.tensor_copy(out=st_bf[:, k, 0:fo],
                                              in_=ps[:, 0:fo])
                        nc.vector.tensor_copy(out=st_f32[:, k, fo // 2 : fo // 2 + 1],
                                              in_=ps[:, fo : fo + 1])
                    nc.sync.dma_start(
                        out=tbl_v[o][:, g0 : g0 + gsz, 0:he],
                        in_=st[:, 0 : gsz * he])
            # own a_dst pass (layer 1)
            adsb = resp.tile([128, nb], f32, name="adsb0", tag="adsb")
            for g0 in range(0, nb, PGA):
                gsz = min(PGA, nb - g0)
                xt = pa.tile([f_in0, PGA * 128], f32, name="xt", tag="xt")
                nc.sync.dma_start(out=xt[:, 0 : gsz * 128],
                                  in_=xTo_in[:, g0 * 128 : (g0 + gsz) * 128])
                for k in range(gsz):
                    ps = psum_a.tile([128, we], f32, name="pa_h", tag="pa_h")
                    nc.tensor.matmul(out=ps[:, 0:1],
                                     lhsT=xt[:, k * 128 : (k + 1) * 128],
                                     rhs=wsb[0][:, fo + 1 : fo + 2],
                                     start=True, stop=True)
                    nc.scalar.copy(out=adsb[:, g0 + k : g0 + k + 1],
                                   in_=ps[:, 0:1])
            adst_v = adst_d[0][:].rearrange("(p b) c -> p b c", p=128)
            nc.sync.dma_start(out=adst_v[:, :, 0:1], in_=adsb[:].unsqueeze(2))

            # ------------- edge phase ------------------------------------
            def edge_phase(li):
                L = LAYERS[li]
                he, fo, rw = L["he"], L["f_out"], L["rw"]
                res = alloc_res(li)
                ctmax = scw * max(K)
                first_s = min(s for s in range(NRANGE) if K[s] > 0)
                last_s = max(s for s in range(NRANGE) if K[s] > 0)
                for qd in range(nw // scw):
                    pws = [None] * scw
                    for s in range(NRANGE):
                        if K[s] == 0:
                            continue
                        ct = scw * K[s]
                        t0 = qd * (scw * k_tot) + scw * cumK[s]
                        g = stage.tile([128, ctmax * he], i16, name="g", tag="g")
                        gv = g[:, 0 : ct * he].rearrange("p (t c) -> p t c", c=he)
                        ad = stage.tile([128, ctmax], f32, name="ad", tag="ad")
                        # split gathers: <=6 tiles (768 descs) per inst to fit
                        # the SWDGE descriptor-ring carveout (1024 descs)
                        for c0 in range(0, ct, 6):
                            c1 = min(c0 + 6, ct)
                            _dma_gather_raw(
                                nc.gpsimd, gv[:, c0:c1, :],
                                tbl[s * rng_rows : (s + 1) * rng_rows, 0:he],
                                src16[:, 8 * (t0 + c0) : 8 * (t0 + c1)],
                                (c1 - c0) * 128, he, 128,
                                queue_num=0,
                            )
                            _dma_gather_raw(
                                nc.gpsimd, ad[:, c0:c1].unsqueeze(2),
                                adst_d[li][:, 0:1],
                                aidx16[:, 8 * (t0 + c0) : 8 * (t0 + c1)],
                                (c1 - c0) * 128, 1, 64,
                                queue_num=0,
                            )
                        # scores
                        s_t = stage.tile([128, ctmax], f32, name="s_t", tag="s_t")
                        tmp = stage.tile([128, ctmax], f32, name="tmp", tag="tmp")
                        asrc = gv[:, :, fo : fo + 2].bitcast(f32)
                        nc.vector.tensor_tensor(
                            out=s_t[:, 0:ct].unsqueeze(2), in0=asrc,
                            in1=ad[:, 0:ct].unsqueeze(2), op=OP.add)
                        nc.vector.tensor_scalar_mul(out=tmp[:, 0:ct],
                                                    in0=s_t[:, 0:ct], scalar1=0.2)
                        nc.vector.tensor_tensor(out=s_t[:, 0:ct],
                                                in0=s_t[:, 0:ct],
                                                in1=tmp[:, 0:ct], op=OP.max)
                        nc.scalar.activation(out=tmp[:, 0:ct], in_=s_t[:, 0:ct],
                                             func=AF.Exp)
                        # Gs = [p*h | p]
                        gs = stage.tile([128, ctmax * rw], f32, name="gs", tag="gs")
                        gsv = gs[:, 0 : ct * rw].rearrange("p (t c) -> p t c", c=rw)
                        nc.vector.tensor_tensor(
                            out=gsv[:, :, 0:fo],
                            in0=gv[:, :, 0:fo].bitcast(bf16),
                            in1=tmp[:, 0:ct].unsqueeze(2).to_broadcast(
                                [128, ct, fo]),
                            op=OP.mult)
                        nc.vector.tensor_copy(out=gsv[:, :, fo : fo + 1],
                                              in_=tmp[:, 0:ct].unsqueeze(2))
                        # one-hot S
                        S = stage.tile([128, ctmax * WSL], f32, name="S", tag="S")
                        nc.vector.tensor_tensor(
                            out=S[:, 0 : ct * WSL],
                            in0=dstrel[:, t0 : t0 + ct].unsqueeze(2)
                                .to_broadcast([128, ct, WSL]),
                            in1=iota_sb[:].unsqueeze(1).to_broadcast(
                                [128, ct, WSL]),
                            op=OP.is_equal)
                        for tt in range(ct):
                            w4 = tt // K[s]
                            tin = tt - w4 * K[s]
                            if s == first_s and tin == 0:
                                pws[w4] = psum_w.tile([WSL, rw], f32,
                                                      name="pwin", tag="pwin")
                            pw = pws[w4]
                            nc.tensor.matmul(
                                out=pw[:],
                                lhsT=S[:, tt * WSL : (tt + 1) * WSL],
                                rhs=gs[:, tt * rw : (tt + 1) * rw],
                                start=(s == first_s and tin == 0),
                                stop=(s == last_s and tin == K[s] - 1),
                            )
                            if s == last_s and tin == K[s] - 1:
                                w = qd * scw + w4
                                p0 = (w % 2) * WSL
                                nc.vector.tensor_copy(
                                    out=res[p0 : p0 + WSL,
                                            (w // 2) * rw : (w // 2) * rw + rw],
                                    in_=pw[:])
                # normalize + bias (+relu) in bulk
                rv = res[:].rearrange("p (b c) -> p b c", c=rw)
                den = stage.tile([128, nb], f32, name="den", tag="den")
                nc.vector.tensor_scalar(out=den[:], in0=rv[:, :, fo],
                                        scalar1=1e-30, scalar2=None, op0=OP.add)
                rec = stage.tile([128, nb], f32, name="rec", tag="rec")
                nc.vector.reciprocal(out=rec[:], in_=den[:])
                nc.vector.tensor_tensor(
                    out=rv[:, :, 0:fo], in0=rv[:, :, 0:fo],
                    in1=rec[:].unsqueeze(2).to_broadcast([128, nb, fo]),
                    op=OP.mult)
                nc.vector.tensor_tensor(
                    out=rv[:, :, 0:fo], in0=rv[:, :, 0:fo],
                    in1=bsb[li][:].unsqueeze(1).to_broadcast([128, nb, fo]),
                    op=OP.add)
                if L["relu"]:
                    nc.vector.tensor_scalar_max(out=rv[:, :, 0:fo],
                                                in0=rv[:, :, 0:fo], scalar1=0.0)
                nc.sync.dma_start(out=outs[li][:, :], in_=res[:])

            # ------------- phase A for layers 2/3 ------------------------
            def phase_a_next(li):
                L = LAYERS[li]
                he, we, fo = L["he"], L["we"], L["f_out"]
                rwp = LAYERS[li - 1]["rw"]
                fop = LAYERS[li - 1]["f_out"]
                act = res_t[li - 1]
                adsb = resp.tile([128, nb], f32, name=f"adsb{li}", tag="adsb")
                hxo_v = hxo_c[li][:].rearrange("(p b) c -> p (b c)", p=128)
                for g0 in range(0, nb, PGA):
                    gsz = min(PGA, nb - g0)
                    st = pa.tile([128, PGA * he], i16, name="st2", tag="st1")
                    st_bf = st[:].bitcast(bf16).rearrange("p (j c) -> p j c", c=he)
                    st_f32 = st[:].bitcast(f32).rearrange("p (j c) -> p j c",
                                                          c=he // 2)
                    for k in range(gsz):
                        b = g0 + k
                        pt = psum_a.tile([L["f_in"], 128], f32, name="pa_t",
                                         tag="pa_t")
                        nc.tensor.transpose(
                            out=pt[:], in_=act[:, b * rwp : b * rwp + fop],
                            identity=ident[:])
                        at = pa.tile([L["f_in"], 128], f32, name="at", tag="at")
                        nc.vector.tensor_copy(out=at[:], in_=pt[:])
                        ps = psum_a.tile([128, we], f32, name="pa_h", tag="pa_h")
                        nc.tensor.matmul(out=ps[:], lhsT=at[:], rhs=wsb[li][:],
                                         start=True, stop=True)
                        nc.vector.tensor_copy(out=st_bf[:, k, 0:fo],
                                              in_=ps[:, 0:fo])
                        nc.vector.tensor_copy(out=st_f32[:, k, fo // 2 : fo // 2 + 1],
                                              in_=ps[:, fo : fo + 1])
                        nc.scalar.copy(out=adsb[:, b : b + 1],
                                       in_=ps[:, fo + 1 : fo + 2])
                    nc.sync.dma_start(out=hxo_v[:, g0 * he : (g0 + gsz) * he],
                                      in_=st[:, 0 : gsz * he])
                adst_v = adst_d[li][:].rearrange("(p b) c -> p b c", p=128)
                nc.sync.dma_start(out=adst_v[:, :, 0:1], in_=adsb[:].unsqueeze(2))
                nc.gpsimd.collective_compute(
                    "AllGather", mybir.AluOpType.bypass, replica_groups=rg,
                    ins=[hxo_c[li].opt()], outs=[hxf_c[li].opt()])
                # re-pitch compact rows into the 256B-pitch table
                # (split: AP dims must fit 16-bit ISA fields)
                nchunk = (npadg + 49151) // 49152
                rows = (npadg + nchunk - 1) // nchunk
                for r0 in range(0, npadg, rows):
                    r1 = min(r0 + rows, npadg)
                    nc.sync.dma_start(out=tbl[r0:r1, 0:he],
                                      in_=hxf_c[li][r0:r1, :])

            edge_phase(0)
            phase_a_next(1)
            edge_phase(1)
            phase_a_next(2)
            edge_phase(2)

    nc.compile()
    return nc


# --------------------------------------------------------------------------
# Entry point
# --------------------------------------------------------------------------

def _run(inputs, num_cores=N_CORES, runner=None):
    x = np.asarray(inputs["x"])
    edge_index = np.asarray(inputs["edge_index"])
    meta, cores, xT, xT_owns, slot_of = _prep(x, edge_index)

    key = (x.shape, edge_index.shape, num_cores, meta["k_tot"])
    if key not in _cache:
        _cache[key] = _build_program(meta, num_cores)
    nc = _cache[key]

    iota = np.tile(np.arange(WSL, dtype=np.float32), (128, 1))
    wext = [
        _make_wext(np.asarray(inputs["W1"]), np.asarray(inputs["as1"]),
                   np.asarray(inputs["ad1"])),
        _make_wext(np.asarray(inputs["W2"]), np.asarray(inputs["as2"]),
                   np.asarray(inputs["ad2"])),
        _make_wext(np.asarray(inputs["W3"]), np.asarray(inputs["as3"]),
                   np.asarray(inputs["ad3"])),
    ]
    bias = [np.tile(np.asarray(inputs[f"b{i}"])[None, :], (128, 1)).astype(np.float32)
            for i in (1, 2, 3)]

    in_maps = []
    for c in range(num_cores):
        m = dict(xT=xT, xTo=xT_owns[c], iota=iota)
        for li in range(3):
            m[f"wext{li}"] = wext[li]
            m[f"bias{li}"] = bias[li]
        for nm in ("src16", "aidx16", "dstrel"):
            m[nm] = cores[c][nm]
        in_maps.append(m)

    if runner is None:
        from concourse.bass_utils import run_bass_kernel_spmd
        res = run_bass_kernel_spmd(nc, in_maps, list(range(num_cores)))
        results = res.results
    else:
        results = runner(nc, in_maps)

    # ---- host-side unshard ----
    n = meta["n"]
    d_own = meta["d_own"]
    nb = meta["nb"]
    nodes = np.arange(n)
    o = nodes // d_own
    s = slot_of
    p = s % 128
    b = s // 128

    def gather_out(name, rw, fo):
        full = np.empty((n, fo), dtype=np.float32)
        for c in range(N_CORES):
            r = results[c][name].reshape(128, nb, rw)
            mask = o == c
            full[mask] = r[p[mask], b[mask], 0:fo]
        return full

    h1 = gather_out("out0", 65, 64)
    h2 = gather_out("out1", 65, 64)
    o3 = gather_out("out2", 33, 32)
    return o3, h1, h2


def kernel(**inputs):
    return _run(inputs)


# revision 18
# speedup vs baseline: 1.7322x; 1.7322x over previous
"""Bass/Trainium2 kernel for 3-layer GAT (nn_GAT_90744069030460).

Strategy (8-core SPMD, graph/data parallel):
  - Nodes sharded contiguously across 8 cores by dst owner.  Each core's dst
    nodes are assigned to 64-slot "windows" (host balances per-src-range edge
    loads across windows).
  - Per layer a packed gather table (256B-pitch rows: [h bf16 x fo | a_src
    f32 | pad]) is built for all nodes: layer 1 computes it locally from the
    replicated x; layers 2-3 compute own rows, AllGather a compact copy, and
    re-pitch locally.
  - Edge phase: edges grouped by (super-chunk of 4 windows) x (src range).
    Ranges are owner-pairs so int16 dma_gather indices reach.  One bulk
    dma_gather per (super-chunk, range) fetches h+a_src rows; per-edge a_dst
    comes from the one-hot S times the window's a_dst vector (DVE multiply +
    reduce).  Per-edge p = exp(leakyrelu(a_src + a_dst)) (segment-max-free softmax, safe in f32).  Segment sums go through
    one-hot selection matmuls into per-window PSUM accumulators; the softmax
    denominator comes from an extra column holding p.
  - Normalize (divide by denominator) + bias + relu in bulk on DVE.

Host-side numpy does all graph prep (window balancing, padding, index
arrays); the device only runs dense/regular work.
"""

import sys

sys.path.insert(0, "/opt/trn_rl_repo")

import heapq
import numpy as np

N_CORES = 8
WSL = 64   # dst slots per window
PGA = 4    # phase-A blocks per group
SCW = 4    # windows per super-chunk
NRANGE = 4  # src ranges (owner pairs)

_cache = {}


# --------------------------------------------------------------------------
# Host-side graph preprocessing
# --------------------------------------------------------------------------

def _balance_windows(degv, nw):
    """Assign local dsts to (window, pos) slots.  degv: [d_own, NRANGE] edge
    counts per src range.  Balances the per-range max load across windows,
    capacity WSL dsts per window."""
    d_own = degv.shape[0]
    tot = degv.sum(axis=1)
    order = np.argsort(-tot, kind="stable")
    loads = np.zeros((nw, NRANGE), dtype=np.int64)
    counts = np.zeros(nw, dtype=np.int64)
    slot = np.empty(d_own, dtype=np.int64)
    target = np.maximum(degv.sum(axis=0) / nw, 1.0)
    for i in order:
        cand = ((loads + degv[i][None, :]) / target[None, :]).max(axis=1)
        cand[counts >= WSL] = np.inf
        w = int(np.argmin(cand))
        slot[i] = w * WSL + counts[w]
        counts[w] += 1
        loads[w] += degv[i]
    return slot


def _prep(x, edge_index):
    n, f_in = x.shape
    assert n % N_CORES == 0
    d_own = n // N_CORES
    nb = (d_own + 127) // 128
    d_pad = nb * 128
    nw = 2 * nb
    nbg = N_CORES * nb
    npadg = N_CORES * d_pad
    range_rows = 2 * d_pad
    assert range_rows <= 32768

    src = np.concatenate([edge_index[0].astype(np.int64), np.arange(n, dtype=np.int64)])
    dst = np.concatenate([edge_index[1].astype(np.int64), np.arange(n, dtype=np.int64)])
    owner = dst // d_own
    loc = dst - owner * d_own
    src_owner = src // d_own
    src_range = src_owner // 2

    slot_of = np.empty(n, dtype=np.int64)
    for c in range(N_CORES):
        m = owner == c
        degv = np.zeros((d_own, NRANGE), dtype=np.int64)
        np.add.at(degv, (loc[m], src_range[m]), 1)
        slot_of[c * d_own : (c + 1) * d_own] = _balance_windows(degv, nw)

    s_e = slot_of[dst]
    w_e = s_e // WSL

    # per-range K (tiles per (window, range)), global max across cores
    K = np.zeros(NRANGE, dtype=np.int64)
    percore = []
    for c in range(N_CORES):
        idx = np.nonzero(owner == c)[0]
        cnt = np.zeros((nw, NRANGE), dtype=np.int64)
        np.add.at(cnt, (w_e[idx], src_range[idx]), 1)
        K = np.maximum(K, (cnt.max(axis=0) + 127) // 128)
        percore.append((idx, cnt))
    K = np.maximum(K, 0)
    cumK = np.zeros(NRANGE + 1, dtype=np.int64)
    cumK[1:] = np.cumsum(K)
    k_tot = int(cumK[-1])
    scw = min(SCW, nw)
    assert nw % scw == 0
    T = nw * k_tot

    # table row of node (o, s): r = o*d_pad + (s%128)*nb + s//128
    s_src = slot_of[src]
    row_src = src_owner * d_pad + (s_src % 128) * nb + s_src // 128
    rel_src = row_src - src_range * range_rows
    assert rel_src.min() >= 0 and rel_src.max() < range_rows

    cores = []
    for c in range(N_CORES):
        idx, cnt = percore[c]
        we_ = w_e[idx]
        se_ = src_range[idx]
        order = np.argsort(we_ * NRANGE + se_, kind="stable")
        idx = idx[order]
        we_ = we_[order]
        se_ = se_[order]
        # rank within (window, range)
        starts = np.zeros(nw * NRANGE, dtype=np.int64)
        flatcnt = cnt.reshape(-1)
        starts[1:] = np.cumsum(flatcnt)[:-1]
        rank = np.arange(idx.shape[0], dtype=np.int64) - starts[we_ * NRANGE + se_]
        t = ((we_ // scw) * (scw * k_tot) + scw * cumK[se_]
             + (we_ % scw) * K[se_] + rank // 128)
        p = rank % 128

        src16 = np.zeros((128, 8 * T), dtype=np.int16)
        dstrel = np.full((128, T), float(WSL), dtype=np.float32)

        sev = s_e[idx]
        sval = rel_src[idx].astype(np.int16)
        col = t * 8 + p // 16
        prow = (p % 16).astype(np.int64)
        for r in range(8):
            src16[16 * r + prow, col] = sval
        dstrel[p, t] = (sev % WSL).astype(np.float32)
        cores.append(dict(src16=src16, dstrel=dstrel))

    # node_at[o, slot]
    node_at = np.full((N_CORES, d_pad), -1, dtype=np.int64)
    allnodes = np.arange(n, dtype=np.int64)
    node_at[allnodes // d_own, slot_of] = allnodes

    # xT [f_in, npadg]: col (o*nb+b)*128+q = x[node_at[o, b*128+q]]
    j = np.arange(nbg)
    q = np.arange(128)
    o_j = (j // nb)[:, None]
    s_jq = ((j % nb) * 128)[:, None] + q[None, :]
    nid = node_at[o_j, s_jq]
    x_rows = np.where(nid[:, :, None] >= 0, x[np.clip(nid, 0, None), :], 0.0)
    xT = np.ascontiguousarray(x_rows.reshape(nbg * 128, f_in).T.astype(np.float32))

    # per-core xT_own [f_in, d_pad]
    xT_owns = []
    for c in range(N_CORES):
        nid = node_at[c]
        xr = np.where(nid[:, None] >= 0, x[np.clip(nid, 0, None), :], 0.0)
        xT_owns.append(np.ascontiguousarray(xr.T.astype(np.float32)))

    meta = dict(n=n, f_in=f_in, d_own=d_own, nb=nb, d_pad=d_pad, nw=nw,
                nbg=nbg, npadg=npadg, range_rows=range_rows, scw=scw,
                K=tuple(int(v) for v in K), k_tot=k_tot, t_tiles=T)
    return meta, cores, xT, xT_owns, slot_of


def _make_wext(W, a_s, a_d):
    f_in, f_out = W.shape
    m = np.zeros((f_in, f_out + 4), dtype=np.float32)
    m[:, 0:f_out] = W
    m[:, f_out] = W @ a_s
    m[:, f_out + 1] = W @ a_d
    return m


# --------------------------------------------------------------------------
# Device program
# --------------------------------------------------------------------------

def _dma_gather_raw(eng, out_ap, in_ap, idxs_ap, num_idxs, elem_size,
                    elem_step, queue_num):
    """bass.dma_gather (non-transpose, DRAM source) without the 256B
    elem-size assert — the Q7 ucode only needs the row *stride* to be a
    256B multiple; elem_size is arbitrary (packets are min(elem, 16K))."""
    import concourse.bass as bass
    import concourse.mybir as mybir
    from concourse.bass import MemorySpace
    from concourse._compat import exact_div

    assert idxs_ap.dtype == mybir.dt.int16
    assert in_ap.space == MemorySpace.DRAM
    assert in_ap.dtype == out_ap.dtype
    assert in_ap.ap[-1][1] == out_ap.ap[-1][1] == elem_size
    assert in_ap.ap[0][0] == elem_step
    assert num_idxs % 128 == 0
    assert out_ap.ap[0][1] * out_ap.ap[1][1] == num_idxs
    stride_bytes = elem_step * mybir.dt.size(in_ap.dtype)
    stride_bytes_256 = exact_div(stride_bytes, 256)
    assert 0 < stride_bytes_256 < 256
    _in_ap = eng.lower_ap_dma(in_ap, for_custom_bir_dma=True)
    _idxs_ap = eng.lower_ap(idxs_ap)
    _out_ap = eng.lower_ap(out_ap)
    return eng.add_instruction(
        mybir.InstDMAGatherAnt(
            name=eng.bass.get_next_instruction_name(),
            ins=[*_in_ap, _idxs_ap,
                 eng.lower_val_access(eng.to_reg(num_idxs))],
            outs=[_out_ap],
            transpose=False,
            num_idxs=num_idxs,
            elem_size=elem_size,
            stride_bytes_256=stride_bytes_256,
            gen_mode=0,
            single_packet=True,
            queue_num=queue_num,
            sbuf_tokens_per_rank=0,
            sbuf_free_dim_per_rank=0,
            sbuf_free_dim_pad_per_rank=0,
            sbuf_byte_offset=0,
        )
    )


def _patch_tile_gather_lanes():
    """Make Tile's DMASW semaphore-lane assignment queue-aware for
    InstDMAGatherAnt so gathers can spread across the 4 Q7 queue pairs
    (each queue keeps a dedicated lane -> per-lane FIFO stays sound)."""
    import concourse.mybir as mybir
    import concourse.tile_sem_assignment as tsa
    if getattr(tsa.TileClockTick, "_gat_patched", False):
        return
    orig = tsa.TileClockTick._assign_tick

    def patched(self, inst):
        if isinstance(inst, mybir.InstDMAGatherAnt):
            save = self.next_sw_dma_idx
            self.next_sw_dma_idx = inst.queue_num % self.swdge_sem_count
            orig(self, inst)
            self.next_sw_dma_idx = save
            return
        orig(self, inst)

    tsa.TileClockTick._assign_tick = patched
    tsa.TileClockTick._gat_patched = True


def _build_program(meta, num_cores, debug=False):
    import concourse.bass as bass
    import concourse.bacc as bacc
    import concourse.tile as tile
    import concourse.mybir as mybir
    from concourse.masks import make_identity

    _patch_tile_gather_lanes()

    f32 = mybir.dt.float32
    bf16 = mybir.dt.bfloat16
    i16 = mybir.dt.int16
    OP = mybir.AluOpType
    AF = mybir.ActivationFunctionType

    nb = meta["nb"]
    nbg = meta["nbg"]
    npadg = meta["npadg"]
    d_pad = meta["d_pad"]
    nw = meta["nw"]
    rng_rows = meta["range_rows"]
    scw = meta["scw"]
    K = meta["K"]
    k_tot = meta["k_tot"]
    T = meta["t_tiles"]
    f_in0 = meta["f_in"]
    cumK = [0]
    for v in K:
        cumK.append(cumK[-1] + v)

    # layer params: he = packed row cols used (int16), rw = matmul rhs width
    LAYERS = [
        dict(f_in=f_in0, f_out=64, relu=True),
        dict(f_in=64, f_out=64, relu=True),
        dict(f_in=64, f_out=32, relu=False),
    ]
    for L in LAYERS:
        L["he"] = L["f_out"] + 2      # h bf16 cols + a_src f32 (2 int16 cols)
        L["rw"] = L["f_out"] + 1      # rhs: p*h cols + p col (denominator)
        L["we"] = L["f_out"] + 4      # W_ext: W | W@as | W@ad | pad

    nc = bacc.Bacc("TRN2", target_bir_lowering=False, debug=debug,
                   num_devices=num_cores, num_swdge_queues=4)

    # ---- I/O ----
    xT_in = nc.dram_tensor("xT", [f_in0, npadg], f32, kind="ExternalInput")
    xTo_in = nc.dram_tensor("xTo", [f_in0, d_pad], f32, kind="ExternalInput")
    src16_in = nc.dram_tensor("src16", [128, 8 * T], i16, kind="ExternalInput")
    dstrel_in = nc.dram_tensor("dstrel", [128, T], f32, kind="ExternalInput")
    iota_in = nc.dram_tensor("iota", [128, WSL], f32, kind="ExternalInput")
    w_in = {li: nc.dram_tensor(f"wext{li}", [L["f_in"], L["we"]], f32,
                               kind="ExternalInput")
            for li, L in enumerate(LAYERS)}
    b_in = {li: nc.dram_tensor(f"bias{li}", [128, LAYERS[li]["f_out"]], f32,
                               kind="ExternalInput") for li in range(3)}
    outs = {li: nc.dram_tensor(f"out{li}", [128, nb * LAYERS[li]["rw"]], f32,
                               kind="ExternalOutput") for li in range(3)}

    rg = [list(range(num_cores))]

    with tile.TileContext(nc) as tc:
        with (
            tc.tile_pool(name="const", bufs=1) as constp,
            tc.tile_pool(name="edat", bufs=1) as edat,
            tc.tile_pool(name="res", bufs=1) as resp,
            tc.tile_pool(name="stage", bufs=3) as stage,
            tc.tile_pool(name="pa", bufs=3) as pa,
            tc.tile_pool(name="pw", bufs=4, space="PSUM") as psum_w,
            tc.tile_pool(name="pah", bufs=2, space="PSUM") as psum_a,
            tc.tile_pool(name="dram", bufs=1, space="DRAM") as dramp,
        ):
            ident = constp.tile([128, 128], f32, name="ident")
            make_identity(nc, ident[:])
            iota_sb = constp.tile([128, WSL], f32, name="iota_sb")
            nc.sync.dma_start(out=iota_sb[:], in_=iota_in[:, :])
            wsb = {}
            bsb = {}
            for li, L in enumerate(LAYERS):
                wsb[li] = constp.tile([L["f_in"], L["we"]], f32, name=f"w{li}")
                nc.sync.dma_start(out=wsb[li][:], in_=w_in[li][:, :])
                bsb[li] = constp.tile([128, L["f_out"]], f32, name=f"b{li}")
                nc.sync.dma_start(out=bsb[li][:], in_=b_in[li][:, :])

            src16 = edat.tile([128, 8 * T], i16, name="src16")
            nc.sync.dma_start(out=src16[:], in_=src16_in[:, :])
            dstrel = edat.tile([128, T], f32, name="dstrel")
            nc.sync.dma_start(out=dstrel[:], in_=dstrel_in[:, :])

            # ---- DRAM scratch ----
            tbl = dramp.tile([npadg, 128], i16, name="tbl")  # packed gather table
            hxo_c = {1: dramp.tile([d_pad, LAYERS[1]["he"]], i16, name="hxo2"),
                     2: dramp.tile([d_pad, LAYERS[2]["he"]], i16, name="hxo3")}
            hxf_c = {1: dramp.tile([npadg, LAYERS[1]["he"]], i16, name="hxf2"),
                     2: dramp.tile([npadg, LAYERS[2]["he"]], i16, name="hxf3")}

            res_t = {}

            def alloc_res(li):
                res_t[li] = resp.tile([128, nb * LAYERS[li]["rw"]], f32,
                                      name=f"res{li}", tag="res",
                                      padded_shape=[128, nb * 65])
                return res_t[li]

            # ------------- phase A: layer 1 (all nodes -> pitched tbl) -----
            L = LAYERS[0]
            he, we, fo = L["he"], L["we"], L["f_out"]
            tbl_v = tbl[:].rearrange("(o p b) c -> o p b c", o=num_cores, p=128)
            for o in range(num_cores):
                for g0 in range(0, nb, PGA):
                    gsz = min(PGA, nb - g0)
                    xt = pa.tile([f_in0, PGA * 128], f32, name="xt", tag="xt")
                    nc.sync.dma_start(
                        out=xt[:, 0 : gsz * 128],
                        in_=xT_in[:, (o * nb + g0) * 128 : (o * nb + g0 + gsz) * 128])
                    st = pa.tile([128, PGA * he], i16, name="st1", tag="st1")
                    st_bf = st[:].bitcast(bf16).rearrange("p (j c) -> p j c", c=he)
                    st_f32 = st[:].bitcast(f32).rearrange("p (j c) -> p j c",
                                                          c=he // 2)
                    for k in range(gsz):
                        ps = psum_a.tile([128, we], f32, name="pa_h", tag="pa_h")
                        nc.tensor.matmul(out=ps[:],
                                         lhsT=xt[:, k * 128 : (k + 1) * 128],
                                         rhs=wsb[0][:], start=True, stop=True)
                        nc.vector.tensor_copy(out=st_bf[:, k, 0:fo],
                                              in_=ps[:, 0:fo])
                        nc.vector.tensor_copy(out=st_f32[:, k, fo // 2 : fo // 2 + 1],
                                              in_=ps[:, fo : fo + 1])
                    nc.sync.dma_start(
                        out=tbl_v[o][:, g0 : g0 + gsz, 0:he],
                        in_=st[:, 0 : gsz * he])
            # own a_dst pass (layer 1)
            adsb = resp.tile([128, nb], f32, name="adsb0", tag="adsb")
            for g0 in range(0, nb, PGA):
                gsz = min(PGA, nb - g0)
                xt = pa.tile([f_in0, PGA * 128], f32, name="xt", tag="xt")
                nc.sync.dma_start(out=xt[:, 0 : gsz * 128],
                                  in_=xTo_in[:, g0 * 128 : (g0 + gsz) * 128])
                for k in range(gsz):
                    ps = psum_a.tile([128, we], f32, name="pa_h", tag="pa_h")
                    nc.tensor.matmul(out=ps[:, 0:1],
                                     lhsT=xt[:, k * 128 : (k + 1) * 128],
                                     rhs=wsb[0][:, fo + 1 : fo + 2],
                                     start=True, stop=True)
                    nc.scalar.copy(out=adsb[:, g0 + k : g0 + k + 1],
                                   in_=ps[:, 0:1])
            adT0 = resp.tile([nb, 128], f32, name="adT0", tag="adT")
            ptT = psum_a.tile([nb, 128], f32, name="pa_t", tag="pa_t")
            nc.tensor.transpose(out=ptT[:], in_=adsb[:, 0:nb], identity=ident[:])
            nc.vector.tensor_copy(out=adT0[:], in_=ptT[:])
            adw_d = {li: dramp.tile([1, d_pad], f32, name=f"adw{li}")
                     for li in range(3)}
            nc.sync.dma_start(
                out=adw_d[0][:].rearrange("one (b j) -> (one b) j", j=128),
                in_=adT0[:])

            # ------------- edge phase ------------------------------------
            def edge_phase(li):
                L = LAYERS[li]
                he, fo, rw = L["he"], L["f_out"], L["rw"]
                res = alloc_res(li)
                ctmax = scw * max(K)
                first_s = min(s for s in range(NRANGE) if K[s] > 0)
                last_s = max(s for s in range(NRANGE) if K[s] > 0)
                for qd in range(nw // scw):
                    pws = [None] * scw
                    # replicate this super-chunk's a_dst window values to all
                    # partitions (DVE can't partition-broadcast)
                    nhalf = max(scw // 2, 1)
                    adw_q = stage.tile([128, max(scw, 2) * WSL], f32,
                                       name="adw_q", tag="adw_q")
                    for h in range(nhalf):
                        b0 = qd * scw // 2 + h
                        nc.sync.dma_start(
                            out=adw_q[:, h * 128 : (h + 1) * 128],
                            in_=adw_d[li][0:1, b0 * 128 : (b0 + 1) * 128]
                                .to_broadcast([128, 128]))
                    for s in range(NRANGE):
                        if K[s] == 0:
                            continue
                        ct = scw * K[s]
                        t0 = qd * (scw * k_tot) + scw * cumK[s]
                        g = stage.tile([128, ctmax * he], i16, name="g", tag="g")
                        gv = g[:, 0 : ct * he].rearrange("p (t c) -> p t c", c=he)
                        # split gathers: <=6 tiles (768 descs) per inst to fit
                        # the SWDGE descriptor-ring carveout (1024 descs);
                        # rotate across the 4 Q7 queue pairs
                        for pi, c0 in enumerate(range(0, ct, 6)):
                            c1 = min(c0 + 6, ct)
                            _dma_gather_raw(
                                nc.gpsimd, gv[:, c0:c1, :],
                                tbl[s * rng_rows : (s + 1) * rng_rows, 0:he],
                                src16[:, 8 * (t0 + c0) : 8 * (t0 + c1)],
                                (c1 - c0) * 128, he, 128,
                                queue_num=(qd * NRANGE + s + pi) % 4,
                            )
                        # one-hot S (built early: also used for a_dst expand)
                        S = stage.tile([128, ctmax * WSL], f32, name="S", tag="S")
                        nc.vector.tensor_tensor(
                            out=S[:, 0 : ct * WSL],
                            in0=dstrel[:, t0 : t0 + ct].unsqueeze(2)
                                .to_broadcast([128, ct, WSL]),
                            in1=iota_sb[:].unsqueeze(1).to_broadcast(
                                [128, ct, WSL]),
                            op=OP.is_equal)
                        # a_dst per edge = sum_j S[e, j] * a_dst_win[j]
                        s2 = stage.tile([128, ctmax * WSL], f32, name="s2", tag="s2")
                        kh = ct // nhalf
                        for h in range(nhalf):
                            s2v = s2[:, h * kh * WSL : (h + 1) * kh * WSL]\
                                .rearrange("p (w k j) -> p w k j", w=2, j=WSL)
                            Sv = S[:, h * kh * WSL : (h + 1) * kh * WSL]\
                                .rearrange("p (w k j) -> p w k j", w=2, j=WSL)
                            adw = adw_q[:, h * 128 : (h + 1) * 128]\
                                .rearrange("p (w j) -> p w j", j=WSL)\
                                .unsqueeze(2).to_broadcast([128, 2, kh // 2, WSL])
                            nc.vector.tensor_tensor(out=s2v, in0=Sv, in1=adw,
                                                    op=OP.mult)
                        ad = stage.tile([128, ctmax], f32, name="ad", tag="ad")
                        nc.vector.tensor_reduce(
                            out=ad[:, 0:ct],
                            in_=s2[:, 0 : ct * WSL].rearrange(
                                "p (t j) -> p t j", j=WSL),
                            axis=mybir.AxisListType.X, op=OP.add)
                        # scores
                        s_t = stage.tile([128, ctmax], f32, name="s_t", tag="s_t")
                        tmp = stage.tile([128, ctmax], f32, name="tmp", tag="tmp")
                        asrc = gv[:, :, fo : fo + 2].bitcast(f32)
                        nc.vector.tensor_tensor(
                            out=s_t[:, 0:ct].unsqueeze(2), in0=asrc,
                            in1=ad[:, 0:ct].unsqueeze(2), op=OP.add)
                        nc.vector.tensor_scalar_mul(out=tmp[:, 0:ct],
                                                    in0=s_t[:, 0:ct], scalar1=0.2)
                        nc.vector.tensor_tensor(out=s_t[:, 0:ct],
                                                in0=s_t[:, 0:ct],
                                                in1=tmp[:, 0:ct], op=OP.max)
                        nc.scalar.activation(out=tmp[:, 0:ct], in_=s_t[:, 0:ct],
                                             func=AF.Exp)
                        # Gs = [p*h | p]
                        gs = stage.tile([128, ctmax * rw], f32, name="gs", tag="gs")
                        gsv = gs[:, 0 : ct * rw].rearrange("p (t c) -> p t c", c=rw)
                        nc.vector.tensor_tensor(
                            out=gsv[:, :, 0:fo],
                            in0=gv[:, :, 0:fo].bitcast(bf16),
                            in1=tmp[:, 0:ct].unsqueeze(2).to_broadcast(
                                [128, ct, fo]),
                            op=OP.mult)
                        nc.vector.tensor_copy(out=gsv[:, :, fo : fo + 1],
                                              in_=tmp[:, 0:ct].unsqueeze(2))
                        for tt in range(ct):
                            w4 = tt // K[s]
                            tin = tt - w4 * K[s]
                            if s == first_s and tin == 0:
                                pws[w4] = psum_w.tile([WSL, rw], f32,
                                                      name="pwin", tag="pwin")
                            pw = pws[w4]
                            nc.tensor.matmul(
                                out=pw[:],
                                lhsT=S[:, tt * WSL : (tt + 1) * WSL],
                                rhs=gs[:, tt * rw : (tt + 1) * rw],
                                start=(s == first_s and tin == 0),
                                stop=(s == last_s and tin == K[s] - 1),
                            )
                            if s == last_s and tin == K[s] - 1:
                                w = qd * scw + w4
                                p0 = (w % 2) * WSL
                                nc.vector.tensor_copy(
                                    out=res[p0 : p0 + WSL,
                                            (w // 2) * rw : (w // 2) * rw + rw],
                                    in_=pw[:])
                # normalize + bias (+relu) in bulk
                rv = res[:].rearrange("p (b c) -> p b c", c=rw)
                den = stage.tile([128, nb], f32, name="den", tag="den")
                nc.vector.tensor_scalar(out=den[:], in0=rv[:, :, fo],
                                        scalar1=1e-30, scalar2=None, op0=OP.add)
                rec = stage.tile([128, nb], f32, name="rec", tag="rec")
                nc.vector.reciprocal(out=rec[:], in_=den[:])
                nc.vector.tensor_tensor(
                    out=rv[:, :, 0:fo], in0=rv[:, :, 0:fo],
                    in1=rec[:].unsqueeze(2).to_broadcast([128, nb, fo]),
                    op=OP.mult)
                nc.vector.tensor_tensor(
                    out=rv[:, :, 0:fo], in0=rv[:, :, 0:fo],
                    in1=bsb[li][:].unsqueeze(1).to_broadcast([128, nb, fo]),
                    op=OP.add)
                if L["relu"]:
                    nc.vector.tensor_scalar_max(out=rv[:, :, 0:fo],
                                                in0=rv[:, :, 0:fo], scalar1=0.0)
                nc.sync.dma_start(out=outs[li][:, :], in_=res[:])

            # ------------- phase A for layers 2/3 ------------------------
            def phase_a_next(li):
                L = LAYERS[li]
                he, we, fo = L["he"], L["we"], L["f_out"]
                rwp = LAYERS[li - 1]["rw"]
                fop = LAYERS[li - 1]["f_out"]
                act = res_t[li - 1]
                adsb = resp.tile([128, nb], f32, name=f"adsb{li}", tag="adsb")
                hxo_v = hxo_c[li][:].rearrange("(p b) c -> p (b c)", p=128)
                for g0 in range(0, nb, PGA):
                    gsz = min(PGA, nb - g0)
                    st = pa.tile([128, PGA * he], i16, name="st2", tag="st1")
                    st_bf = st[:].bitcast(bf16).rearrange("p (j c) -> p j c", c=he)
                    st_f32 = st[:].bitcast(f32).rearrange("p (j c) -> p j c",
                                                          c=he // 2)
                    for k in range(gsz):
                        b = g0 + k
                        pt = psum_a.tile([L["f_in"], 128], f32, name="pa_t",
                                         tag="pa_t")
                        nc.tensor.transpose(
                            out=pt[:], in_=act[:, b * rwp : b * rwp + fop],
                            identity=ident[:])
                        at = pa.tile([L["f_in"], 128], f32, name="at", tag="at")
                        nc.vector.tensor_copy(out=at[:], in_=pt[:])
                        ps = psum_a.tile([128, we], f32, name="pa_h", tag="pa_h")
                        nc.tensor.matmul(out=ps[:], lhsT=at[:], rhs=wsb[li][:],
                                         start=True, stop=True)
                        nc.vector.tensor_copy(out=st_bf[:, k, 0:fo],
                                              in_=ps[:, 0:fo])
                        nc.vector.tensor_copy(out=st_f32[:, k, fo // 2 : fo // 2 + 1],
                                              in_=ps[:, fo : fo + 1])
                        nc.scalar.copy(out=adsb[:, b : b + 1],
                                       in_=ps[:, fo + 1 : fo + 2])
                    nc.sync.dma_start(out=hxo_v[:, g0 * he : (g0 + gsz) * he],
                                      in_=st[:, 0 : gsz * he])
                adT = resp.tile([nb, 128], f32, name=f"adT{li}", tag="adT")
                ptT = psum_a.tile([nb, 128], f32, name="pa_t", tag="pa_t")
                nc.tensor.transpose(out=ptT[:], in_=adsb[:, 0:nb],
                                    identity=ident[:])
                nc.vector.tensor_copy(out=adT[:], in_=ptT[:])
                nc.sync.dma_start(
                    out=adw_d[li][:].rearrange("one (b j) -> (one b) j", j=128),
                    in_=adT[:])
                nc.gpsimd.collective_compute(
                    "AllGather", mybir.AluOpType.bypass, replica_groups=rg,
                    ins=[hxo_c[li].opt()], outs=[hxf_c[li].opt()])
                # re-pitch compact rows into the 256B-pitch table
                # (split: AP dims must fit 16-bit ISA fields)
                nchunk = (npadg + 49151) // 49152
                rows = (npadg + nchunk - 1) // nchunk
                for r0 in range(0, npadg, rows):
                    r1 = min(r0 + rows, npadg)
                    nc.sync.dma_start(out=tbl[r0:r1, 0:he],
                                      in_=hxf_c[li][r0:r1, :])

            edge_phase(0)
            phase_a_next(1)
            edge_phase(1)
            phase_a_next(2)
            edge_phase(2)

    nc.compile()
    return nc


# --------------------------------------------------------------------------
# Entry point
# --------------------------------------------------------------------------

def _run(inputs, num_cores=N_CORES, runner=None):
    x = np.asarray(inputs["x"])
    edge_index = np.asarray(inputs["edge_index"])
    meta, cores, xT, xT_owns, slot_of = _prep(x, edge_index)

    key = (x.shape, edge_index.shape, num_cores, meta["k_tot"])
    if key not in _cache:
        _cache[key] = _build_program(meta, num_cores)
    nc = _cache[key]

    iota = np.tile(np.arange(WSL, dtype=np.float32), (128, 1))
    wext = [
        _make_wext(np.asarray(inputs["W1"]), np.asarray(inputs["as1"]),
                   np.asarray(inputs["ad1"])),
        _make_wext(np.asarray(inputs["W2"]), np.asarray(inputs["as2"]),
                   np.asarray(inputs["ad2"])),
        _make_wext(np.asarray(inputs["W3"]), np.asarray(inputs["as3"]),
                   np.asarray(inputs["ad3"])),
    ]
    bias = [np.tile(np.asarray(inputs[f"b{i}"])[None, :], (128, 1)).astype(np.float32)
            for i in (1, 2, 3)]

    in_maps = []
    for c in range(num_cores):
        m = dict(xT=xT, xTo=xT_owns[c], iota=iota)
        for li in range(3):
            m[f"wext{li}"] = wext[li]
            m[f"bias{li}"] = bias[li]
        for nm in ("src16", "dstrel"):
            m[nm] = cores[c][nm]
        in_maps.append(m)

    if runner is None:
        from concourse.bass_utils import run_bass_kernel_spmd
        res = run_bass_kernel_spmd(nc, in_maps, list(range(num_cores)))
        results = res.results
    else:
        results = runner(nc, in_maps)

    # ---- host-side unshard ----
    n = meta["n"]
    d_own = meta["d_own"]
    nb = meta["nb"]
    nodes = np.arange(n)
    o = nodes // d_own
    s = slot_of
    p = s % 128
    b = s // 128

    def gather_out(name, rw, fo):
        full = np.empty((n, fo), dtype=np.float32)
        for c in range(N_CORES):
            r = results[c][name].reshape(128, nb, rw)
            mask = o == c
            full[mask] = r[p[mask], b[mask], 0:fo]
        return full

    h1 = gather_out("out0", 65, 64)
    h2 = gather_out("out1", 65, 64)
    o3 = gather_out("out2", 33, 32)
    return o3, h1, h2


def kernel(**inputs):
    return _run(inputs)
